# revision 12
# baseline (speedup 1.0000x reference)
import sys, functools, time as _time
sys.path.insert(0, "/opt/trn_rl_repo")

import numpy as np
import ml_dtypes
from contextlib import ExitStack

import concourse.bass as bass
import concourse.tile as tile
from concourse import mybir, bacc
from concourse.bass_utils import run_bass_kernel_spmd

BF16 = ml_dtypes.bfloat16
F32 = mybir.dt.float32
BF = mybir.dt.bfloat16
AF = mybir.ActivationFunctionType

B, T, I, H, C, S = 512, 128, 512, 512, 97, 26
NC = 8
BL = B // NC  # 64 batch rows per core
P = 128
HQ = H // P  # 4
IQ = I // P  # 4
KX = 5       # x-contraction k-tiles: 4 (ctx) + 1 (onehot+bias)
G4 = 4 * H   # 2048


def _body(ctx, tc, t_in, t_out):
    nc = tc.nc
    sing = ctx.enter_context(tc.tile_pool(name="sing", bufs=1))
    work = ctx.enter_context(tc.tile_pool(name="work", bufs=2))
    pw = ctx.enter_context(tc.tile_pool(name="pw", bufs=1))
    psG = ctx.enter_context(tc.tile_pool(name="psG", bufs=1, space="PSUM"))
    psM = ctx.enter_context(tc.tile_pool(name="psM", bufs=2, space="PSUM"))

    # ---------- resident SBUF tensors ----------
    sb_bh = sing.tile([P, BL, I], BF)          # batch_H as [t, b, i]
    nc.sync.dma_start(sb_bh, t_in["bh_nat"][:])
    sb_wih = sing.tile([P, KX, G4], BF)
    for k in range(KX):
        nc.sync.dma_start(sb_wih[:, k, :], t_in["wih"][k])
    sb_whh = sing.tile([P, HQ, G4], BF)
    for k in range(HQ):
        nc.sync.dma_start(sb_whh[:, k, :], t_in["whh"][k])
    sb_whid = sing.tile([P, HQ, H], BF)
    for k in range(HQ):
        nc.sync.dma_start(sb_whid[:, k, :], t_in["whid"][k])
    sb_wgen = sing.tile([P, HQ, C], BF)
    for k in range(HQ):
        nc.sync.dma_start(sb_wgen[:, k, :], t_in["wgen"][k])
    sb_wsc = sing.tile([P, HQ, 129], BF)
    for k in range(HQ):
        nc.sync.dma_start(sb_wsc[:, k, :], t_in["wsc"][k])
    sb_bhid = sing.tile([P, HQ], F32)
    for k in range(HQ):
        nc.sync.dma_start(sb_bhid[:, k : k + 1], t_in["bhid"][k])
    sb_bgen = sing.tile([BL, C], F32)
    nc.sync.dma_start(sb_bgen, t_in["bgen"][:])
    sb_id = sing.tile([P, P], BF)
    nc.sync.dma_start(sb_id, t_in["ident"][:])

    sb_feat = sing.tile([P, BL, HQ, T], BF)    # feat.T: [h_in_q, (b, hq, t)]
    adiag = sing.tile([P, 65 * BL + 65], BF)   # alpha diag (t on partitions)
    nc.vector.memset(adiag, 0.0)
    hT = sing.tile([P, HQ, BL], BF)            # h.T state
    nc.vector.memset(hT, 0.0)
    c_sb = sing.tile([BL, H], F32)             # c state, natural
    nc.vector.memset(c_sb, 0.0)

    # ---------- feat precompute: feat.T[h,(b,t)] = W_feat @ batch_H.T ----------
    # bht dram: [iq, 128, BL, T]; chunks of 4 b's (512 cols)
    NB4 = BL // 4
    with tc.tile_pool(name="feed", bufs=1) as feed:
        sb_wfeat = pw.tile([P, IQ, HQ, P], BF, tag="pw_i")
        for k in range(IQ):
            nc.sync.dma_start(sb_wfeat[:, k, :, :], t_in["wfeat"][k])
        for cb in range(NB4):
            rhs = feed.tile([P, IQ, 4, T], BF, tag="bht")
            for k in range(IQ):
                nc.sync.dma_start(rhs[:, k, :, :], t_in["bht"][k, :, 4 * cb : 4 * cb + 4, :])
            for hq in range(HQ):
                ps = psM.tile([P, 4, T], F32, tag="sm")
                for k in range(IQ):
                    nc.tensor.matmul(ps, lhsT=sb_wfeat[:, k, hq, :], rhs=rhs[:, k, :, :],
                                     start=(k == 0), stop=(k == IQ - 1))
                nc.scalar.activation(sb_feat[:, 4 * cb : 4 * cb + 4, hq, :], ps,
                                     AF.Identity, bias=sb_bhid[:, hq : hq + 1], scale=1.0)

    # ---------- scan over S steps ----------
    for s in range(S):
        # hp.T[h,b] = W_hid @ h  (+ b_hid folded into feat)
        hp = work.tile([P, HQ, BL], F32, tag="hp")
        for hq in range(HQ):
            ps = psM.tile([P, BL], F32, tag="sm")
            for k in range(HQ):
                nc.tensor.matmul(ps, lhsT=sb_whid[:, k, hq * P : (hq + 1) * P],
                                 rhs=hT[:, k, :], start=(k == 0), stop=(k == HQ - 1))
            nc.vector.tensor_copy(hp[:, hq, :], ps)

        # tanh(feat + hp) and score e[b,t] accumulated via diagonal trick
        e_ps = psM.tile([BL, T], F32, tag="sm")
        for b in range(BL):
            th = work.tile([P, HQ, T], BF, tag="tanh")
            for hq in range(HQ):
                nc.scalar.activation(th[:, hq, :], sb_feat[:, b, hq, :], AF.Tanh,
                                     bias=hp[:, hq, b : b + 1], scale=1.0)
                nc.tensor.matmul(e_ps, lhsT=sb_wsc[:, hq, 64 - b : 128 - b],
                                 rhs=th[:, hq, :],
                                 start=(b == 0 and hq == 0),
                                 stop=(b == BL - 1 and hq == HQ - 1))

        # softmax over t (free dim)
        mx = work.tile([BL, 1], F32, tag="mx")
        nc.vector.tensor_reduce(mx, e_ps, axis=mybir.AxisListType.X, op=mybir.AluOpType.max)
        nmx = work.tile([BL, 1], F32, tag="nmx")
        nc.vector.tensor_scalar_mul(nmx, mx, -1.0)
        p_sb = work.tile([BL, T], F32, tag="p")
        ssum = work.tile([BL, 1], F32, tag="ssum")
        nc.scalar.activation(p_sb, e_ps, AF.Exp, bias=nmx, scale=1.0, accum_out=ssum)
        rs = work.tile([BL, 1], F32, tag="rs")
        nc.vector.reciprocal(rs, ssum)
        a_bf = work.tile([BL, T], BF, tag="abf")
        nc.vector.tensor_scalar_mul(a_bf, p_sb, rs)
        # alpha.T into diag tile (col 65*b holds alpha[:, b])
        tp = psM.tile([P, BL], BF, tag="sm")
        nc.tensor.transpose(tp, a_bf, sb_id[:BL, :BL])
        ad_v = adiag[:, 0 : 65 * BL].rearrange("p (k r) -> p k r", r=65)
        nc.vector.tensor_copy(ad_v[:, :, 0], tp)

        # ctx[b,i] accumulated via diagonal trick -> psum [BL, I]
        ctx_ps = psM.tile([BL, I], F32, tag="sm")
        for b in range(BL):
            nc.tensor.matmul(ctx_ps, lhsT=adiag[:, 64 * b : 64 * b + 64],
                             rhs=sb_bh[:, b, :], start=(b == 0), stop=(b == BL - 1))
        ctx_bf = work.tile([BL, I], BF, tag="ctxbf")
        nc.vector.tensor_copy(ctx_bf, ctx_ps)

        # x.T assembly: [ctx.T tiles; onehot+bias row tile]
        xT = work.tile([P, KX, BL], BF, tag="xT")
        for k in range(IQ):
            tp2 = psM.tile([P, BL], BF, tag="sm")
            nc.tensor.transpose(tp2, ctx_bf[:, k * P : (k + 1) * P], sb_id[:BL, :BL])
            nc.vector.tensor_copy(xT[:, k, :], tp2)
        oht_t = work.tile([P, BL], BF, tag="oht")
        nc.sync.dma_start(oht_t, t_in["oht"][s])
        nc.vector.tensor_copy(xT[:, 4, :], oht_t)

        # gates = x @ W_ih.T + h @ W_hh.T (+biases folded) -> psum [BL, 4H]
        g_ps = psG.tile([BL, G4], F32, tag="g")
        for k in range(KX):
            for nq in range(4):
                nc.tensor.matmul(g_ps[:, nq * H : (nq + 1) * H], lhsT=xT[:, k, :],
                                 rhs=sb_wih[:, k, nq * H : (nq + 1) * H],
                                 start=(k == 0), stop=False)
        for k in range(HQ):
            for nq in range(4):
                nc.tensor.matmul(g_ps[:, nq * H : (nq + 1) * H], lhsT=hT[:, k, :],
                                 rhs=sb_whh[:, k, nq * H : (nq + 1) * H],
                                 start=False, stop=(k == HQ - 1))

        # LSTM pointwise (gate order i,f,g,o)
        i_s = pw.tile([BL, H], F32, tag="pw_i")
        f_s = pw.tile([BL, H], F32, tag="pw_f")
        g_t = pw.tile([BL, H], F32, tag="pw_g")
        o_s = pw.tile([BL, H], F32, tag="pw_o")
        nc.scalar.activation(i_s, g_ps[:, 0:H], AF.Sigmoid)
        nc.scalar.activation(f_s, g_ps[:, H : 2 * H], AF.Sigmoid)
        nc.scalar.activation(g_t, g_ps[:, 2 * H : 3 * H], AF.Tanh)
        nc.scalar.activation(o_s, g_ps[:, 3 * H : 4 * H], AF.Sigmoid)
        nc.vector.tensor_mul(f_s, f_s, c_sb)
        nc.vector.tensor_mul(i_s, i_s, g_t)
        nc.vector.tensor_add(c_sb, f_s, i_s)
        nc.scalar.activation(g_t, c_sb, AF.Tanh)
        h_bf = work.tile([BL, H], BF, tag="hbf")
        nc.vector.tensor_mul(h_bf, o_s, g_t)
        # h.T for next step + generator
        for k in range(HQ):
            tp3 = psM.tile([P, BL], BF, tag="sm")
            nc.tensor.transpose(tp3, h_bf[:, k * P : (k + 1) * P], sb_id[:BL, :BL])
            nc.vector.tensor_copy(hT[:, k, :], tp3)

        # generator: probs_s = h @ W_gen.T + b_gen
        pr_ps = psM.tile([BL, C], F32, tag="sm")
        for k in range(HQ):
            nc.tensor.matmul(pr_ps, lhsT=hT[:, k, :], rhs=sb_wgen[:, k, :],
                             start=(k == 0), stop=(k == HQ - 1))
        pr_sb = work.tile([BL, C], F32, tag="prsb")
        nc.vector.tensor_add(pr_sb, pr_ps, sb_bgen)
        nc.sync.dma_start(t_out[s], pr_sb[:])


@functools.cache
def build():
    nc = bacc.Bacc("TRN2", target_bir_lowering=False)
    t_in = {}
    def inp(name, shape, dt):
        t_in[name] = nc.dram_tensor(name, shape, dt, kind="ExternalInput")
    inp("bh_nat", [P, BL, I], BF)
    inp("bht", [IQ, P, BL, T], BF)
    inp("wfeat", [IQ, P, HQ, P], BF)
    inp("wih", [KX, P, G4], BF)
    inp("whh", [HQ, P, G4], BF)
    inp("whid", [HQ, P, H], BF)
    inp("wgen", [HQ, P, C], BF)
    inp("wsc", [HQ, P, 129], BF)
    inp("oht", [S, P, BL], BF)
    inp("bhid", [HQ, P, 1], F32)
    inp("bgen", [BL, C], F32)
    inp("ident", [P, P], BF)
    t_out = nc.dram_tensor("probs", [S, BL, C], F32, kind="ExternalOutput")
    with tile.TileContext(nc) as tc:
        with ExitStack() as ctx:
            _body(ctx, tc, t_in, t_out)
    nc.finalize()
    return nc


def _prep_shared(W_feat, W_hid, b_hid, w_score, W_ih, W_hh, b_ih, b_hh, W_gen, b_gen):
    bf = lambda x: np.ascontiguousarray(x, dtype=BF16)
    d = {}
    wf = W_feat.astype(np.float32)  # [H, I]
    d["wfeat"] = bf(wf.T.reshape(IQ, P, HQ, P))  # wfeat[iq][i][hq][h] = W_feat.T[i,h]
    # x.T rows: 0..511 ctx, 512..608 onehot, row 609 -> ones (bias row)
    wihT = np.zeros((KX * P, G4), np.float32)
    wihT[: I + C] = W_ih.astype(np.float32).T
    wihT[I + C] = b_ih.astype(np.float32) + b_hh.astype(np.float32)
    d["wih"] = bf(wihT.reshape(KX, P, G4))
    d["whh"] = bf(W_hh.astype(np.float32).T.reshape(HQ, P, G4))
    d["whid"] = bf(W_hid.astype(np.float32).T.reshape(HQ, P, H))
    d["wgen"] = bf(W_gen.astype(np.float32).T.reshape(HQ, P, C))
    wsc = np.zeros((HQ, P, 129), np.float32)
    wsc[:, :, 64] = w_score.astype(np.float32).reshape(HQ, P)
    d["wsc"] = bf(wsc)
    d["bhid"] = np.ascontiguousarray(b_hid.astype(np.float32).reshape(HQ, P, 1))
    d["bgen"] = np.ascontiguousarray(
        np.broadcast_to(b_gen.astype(np.float32), (BL, C)))
    d["ident"] = bf(np.eye(P))
    return d


def kernel(batch_H, text, W_feat, W_hid, b_hid, w_score, W_ih, W_hh, b_ih, b_hh,
           W_gen, b_gen, _trace=False):
    batch_H = np.asarray(batch_H, dtype=np.float32)
    text = np.asarray(text)
    shared = _prep_shared(W_feat, W_hid, b_hid, w_score, W_ih, W_hh, b_ih, b_hh,
                          W_gen, b_gen)
    oh = np.zeros((B, S, C), np.float32)
    oh[np.arange(B)[:, None], np.arange(S)[None, :], text.astype(np.int64)] = 1.0

    in_maps = []
    for c in range(NC):
        bh = batch_H[c * BL : (c + 1) * BL]          # [BL, T, I]
        m = dict(shared)
        m["bh_nat"] = np.ascontiguousarray(bh.transpose(1, 0, 2), dtype=BF16)
        m["bht"] = np.ascontiguousarray(
            bh.transpose(2, 0, 1).reshape(IQ, P, BL, T), dtype=BF16)
        ohc = np.zeros((S, P, BL), np.float32)
        ohc[:, :C, :] = oh[c * BL : (c + 1) * BL].transpose(1, 2, 0)
        ohc[:, 97, :] = 1.0      # ones row -> multiplies bias row of wih
        m["oht"] = np.ascontiguousarray(ohc, dtype=BF16)
        in_maps.append(m)

    nc = build()
    kw = dict(trace=True, trace_cores=[0]) if _trace else {}
    _t0 = _time.perf_counter()
    res = run_bass_kernel_spmd(nc, in_maps, core_ids=list(range(NC)), **kw)
    kernel._last_exec_s = _time.perf_counter() - _t0
    if _trace:
        kernel._last_result = res
    out = np.empty((B, S, C), np.float32)
    for c in range(NC):
        out[c * BL : (c + 1) * BL] = res.results[c]["probs"].transpose(1, 0, 2)
    return out


# revision 13
# speedup vs baseline: 1.8163x; 1.8163x over previous
import sys, functools, time as _time
sys.path.insert(0, "/opt/trn_rl_repo")

import numpy as np
import ml_dtypes
from contextlib import ExitStack

import concourse.bass as bass
import concourse.tile as tile
from concourse import mybir, bacc
from concourse.bass_utils import run_bass_kernel_spmd

BF16 = ml_dtypes.bfloat16
F32 = mybir.dt.float32
BF = mybir.dt.bfloat16
AF = mybir.ActivationFunctionType

B, T, I, H, C, S = 512, 128, 512, 512, 97, 26
NC = 8
BL = B // NC  # 64 batch rows per core
P = 128
HQ = H // P  # 4
IQ = I // P  # 4
KX = 5       # x-contraction k-tiles: 4 (ctx) + 1 (onehot+bias)
G4 = 4 * H   # 2048


def _body(ctx, tc, t_in, t_out):
    nc = tc.nc
    sing = ctx.enter_context(tc.tile_pool(name="sing", bufs=1))
    work = ctx.enter_context(tc.tile_pool(name="work", bufs=2))
    pw = ctx.enter_context(tc.tile_pool(name="pw", bufs=1))
    psG = ctx.enter_context(tc.tile_pool(name="psG", bufs=1, space="PSUM"))
    psM = ctx.enter_context(tc.tile_pool(name="psM", bufs=2, space="PSUM"))

    # ---------- resident SBUF tensors ----------
    sb_bh = sing.tile([P, BL, I], BF)          # batch_H as [t, b, i]
    nc.sync.dma_start(sb_bh, t_in["bh_nat"][:])
    sb_wih = sing.tile([P, KX, G4], BF)
    for k in range(KX):
        nc.sync.dma_start(sb_wih[:, k, :], t_in["wih"][k])
    sb_whh = sing.tile([P, HQ, G4], BF)
    for k in range(HQ):
        nc.sync.dma_start(sb_whh[:, k, :], t_in["whh"][k])
    sb_whid = sing.tile([P, HQ, H], BF)
    for k in range(HQ):
        nc.sync.dma_start(sb_whid[:, k, :], t_in["whid"][k])
    sb_wgen = sing.tile([P, HQ, C], BF)
    for k in range(HQ):
        nc.sync.dma_start(sb_wgen[:, k, :], t_in["wgen"][k])
    sb_wsc = sing.tile([P, HQ, 129], BF)
    for k in range(HQ):
        nc.sync.dma_start(sb_wsc[:, k, :], t_in["wsc"][k])
    sb_bhid = sing.tile([P, HQ], F32)
    for k in range(HQ):
        nc.sync.dma_start(sb_bhid[:, k : k + 1], t_in["bhid"][k])
    sb_bgen = sing.tile([BL, C], F32)
    nc.sync.dma_start(sb_bgen, t_in["bgen"][:])
    sb_id = sing.tile([P, P], BF)
    nc.sync.dma_start(sb_id, t_in["ident"][:])

    sb_feat = sing.tile([P, BL, HQ, T], BF)    # feat.T: [h_in_q, (b, hq, t)]
    adiag = sing.tile([P, 65 * BL + 65], BF)   # alpha diag (t on partitions)
    nc.vector.memset(adiag, 0.0)
    hT = sing.tile([P, HQ, BL], BF)            # h.T state
    nc.vector.memset(hT, 0.0)
    c_sb = sing.tile([BL, H], F32)             # c state, natural
    nc.vector.memset(c_sb, 0.0)

    # ---------- feat precompute: feat.T[h,(b,t)] = W_feat @ batch_H.T ----------
    # bht dram: [iq, 128, BL, T]; chunks of 4 b's (512 cols)
    NB4 = BL // 4
    with tc.tile_pool(name="feed", bufs=1) as feed:
        sb_wfeat = pw.tile([P, IQ, HQ, P], BF, tag="pw_i")
        for k in range(IQ):
            nc.sync.dma_start(sb_wfeat[:, k, :, :], t_in["wfeat"][k])
        for cb in range(NB4):
            rhs = feed.tile([P, IQ, 4, T], BF, tag="bht")
            for k in range(IQ):
                nc.sync.dma_start(rhs[:, k, :, :], t_in["bht"][k, :, 4 * cb : 4 * cb + 4, :])
            for hq in range(HQ):
                ps = psM.tile([P, 4, T], F32, tag="sm")
                for k in range(IQ):
                    nc.tensor.matmul(ps, lhsT=sb_wfeat[:, k, hq, :], rhs=rhs[:, k, :, :],
                                     start=(k == 0), stop=(k == IQ - 1))
                nc.scalar.activation(sb_feat[:, 4 * cb : 4 * cb + 4, hq, :], ps,
                                     AF.Identity, bias=sb_bhid[:, hq : hq + 1], scale=1.0)

    # ---------- scan over S steps ----------
    for s in range(S):
        # hp.T[h,b] = W_hid @ h  (+ b_hid folded into feat)
        hp = work.tile([P, HQ, BL], F32, tag="hp")
        for hq in range(HQ):
            ps = psM.tile([P, BL], F32, tag="sm")
            for k in range(HQ):
                nc.tensor.matmul(ps, lhsT=sb_whid[:, k, hq * P : (hq + 1) * P],
                                 rhs=hT[:, k, :], start=(k == 0), stop=(k == HQ - 1))
            nc.vector.tensor_copy(hp[:, hq, :], ps)

        # tanh(feat + hp) and score e[b,t] accumulated via diagonal trick
        e_ps = psM.tile([BL, T], F32, tag="sm")
        for b in range(BL):
            th = work.tile([P, HQ, T], BF, tag="tanh")
            for hq in range(HQ):
                nc.scalar.activation(th[:, hq, :], sb_feat[:, b, hq, :], AF.Tanh,
                                     bias=hp[:, hq, b : b + 1], scale=1.0)
                nc.tensor.matmul(e_ps, lhsT=sb_wsc[:, hq, 64 - b : 128 - b],
                                 rhs=th[:, hq, :],
                                 start=(b == 0 and hq == 0),
                                 stop=(b == BL - 1 and hq == HQ - 1))

        # softmax over t (free dim)
        mx = work.tile([BL, 1], F32, tag="mx")
        nc.vector.tensor_reduce(mx, e_ps, axis=mybir.AxisListType.X, op=mybir.AluOpType.max)
        nmx = work.tile([BL, 1], F32, tag="nmx")
        nc.vector.tensor_scalar_mul(nmx, mx, -1.0)
        p_sb = work.tile([BL, T], F32, tag="p")
        ssum = work.tile([BL, 1], F32, tag="ssum")
        nc.scalar.activation(p_sb, e_ps, AF.Exp, bias=nmx, scale=1.0, accum_out=ssum)
        rs = work.tile([BL, 1], F32, tag="rs")
        nc.vector.reciprocal(rs, ssum)
        a_bf = work.tile([BL, T], BF, tag="abf")
        nc.vector.tensor_scalar_mul(a_bf, p_sb, rs)
        # alpha.T into diag tile (col 65*b holds alpha[:, b])
        tp = psM.tile([P, BL], BF, tag="sm")
        nc.tensor.transpose(tp, a_bf, sb_id[:BL, :BL])
        ad_v = adiag[:, 0 : 65 * BL].rearrange("p (k r) -> p k r", r=65)
        nc.vector.tensor_copy(ad_v[:, :, 0], tp)

        # ctx[b,i] accumulated via diagonal trick -> psum [BL, I]
        ctx_ps = psM.tile([BL, I], F32, tag="sm")
        for b in range(BL):
            nc.tensor.matmul(ctx_ps, lhsT=adiag[:, 64 * b : 64 * b + 64],
                             rhs=sb_bh[:, b, :], start=(b == 0), stop=(b == BL - 1))
        ctx_bf = work.tile([BL, I], BF, tag="ctxbf")
        nc.vector.tensor_copy(ctx_bf, ctx_ps)

        # x.T assembly: [ctx.T tiles; onehot+bias row tile]
        xT = work.tile([P, KX, BL], BF, tag="xT")
        for k in range(IQ):
            tp2 = psM.tile([P, BL], BF, tag="sm")
            nc.tensor.transpose(tp2, ctx_bf[:, k * P : (k + 1) * P], sb_id[:BL, :BL])
            nc.vector.tensor_copy(xT[:, k, :], tp2)
        oht_t = work.tile([P, BL], BF, tag="oht")
        nc.sync.dma_start(oht_t, t_in["oht"][s])
        nc.vector.tensor_copy(xT[:, 4, :], oht_t)

        # gates = x @ W_ih.T + h @ W_hh.T (+biases folded) -> psum [BL, 4H]
        g_ps = psG.tile([BL, G4], F32, tag="g")
        for k in range(KX):
            for nq in range(4):
                nc.tensor.matmul(g_ps[:, nq * H : (nq + 1) * H], lhsT=xT[:, k, :],
                                 rhs=sb_wih[:, k, nq * H : (nq + 1) * H],
                                 start=(k == 0), stop=False)
        for k in range(HQ):
            for nq in range(4):
                nc.tensor.matmul(g_ps[:, nq * H : (nq + 1) * H], lhsT=hT[:, k, :],
                                 rhs=sb_whh[:, k, nq * H : (nq + 1) * H],
                                 start=False, stop=(k == HQ - 1))

        # LSTM pointwise (gate order i,f,g,o)
        i_s = pw.tile([BL, H], F32, tag="pw_i")
        f_s = pw.tile([BL, H], F32, tag="pw_f")
        g_t = pw.tile([BL, H], F32, tag="pw_g")
        o_s = pw.tile([BL, H], F32, tag="pw_o")
        nc.scalar.activation(i_s, g_ps[:, 0:H], AF.Sigmoid)
        nc.scalar.activation(f_s, g_ps[:, H : 2 * H], AF.Sigmoid)
        nc.scalar.activation(g_t, g_ps[:, 2 * H : 3 * H], AF.Tanh)
        nc.scalar.activation(o_s, g_ps[:, 3 * H : 4 * H], AF.Sigmoid)
        nc.vector.tensor_mul(f_s, f_s, c_sb)
        nc.vector.tensor_mul(i_s, i_s, g_t)
        nc.vector.tensor_add(c_sb, f_s, i_s)
        nc.scalar.activation(g_t, c_sb, AF.Tanh)
        h_bf = work.tile([BL, H], BF, tag="hbf")
        nc.vector.tensor_mul(h_bf, o_s, g_t)
        # h.T for next step + generator
        for k in range(HQ):
            tp3 = psM.tile([P, BL], BF, tag="sm")
            nc.tensor.transpose(tp3, h_bf[:, k * P : (k + 1) * P], sb_id[:BL, :BL])
            nc.vector.tensor_copy(hT[:, k, :], tp3)

        # generator: probs_s = h @ W_gen.T + b_gen
        pr_ps = psM.tile([BL, C], F32, tag="sm")
        for k in range(HQ):
            nc.tensor.matmul(pr_ps, lhsT=hT[:, k, :], rhs=sb_wgen[:, k, :],
                             start=(k == 0), stop=(k == HQ - 1))
        pr_sb = work.tile([BL, C], F32, tag="prsb")
        nc.vector.tensor_add(pr_sb, pr_ps, sb_bgen)
        nc.sync.dma_start(t_out[s], pr_sb[:])


@functools.cache
def build():
    nc = bacc.Bacc("TRN2", target_bir_lowering=False)
    t_in = {}
    def inp(name, shape, dt):
        t_in[name] = nc.dram_tensor(name, shape, dt, kind="ExternalInput")
    inp("bh_nat", [P, BL, I], BF)
    inp("bht", [IQ, P, BL, T], BF)
    inp("wfeat", [IQ, P, HQ, P], BF)
    inp("wih", [KX, P, G4], BF)
    inp("whh", [HQ, P, G4], BF)
    inp("whid", [HQ, P, H], BF)
    inp("wgen", [HQ, P, C], BF)
    inp("wsc", [HQ, P, 129], BF)
    inp("oht", [S, P, BL], BF)
    inp("bhid", [HQ, P, 1], F32)
    inp("bgen", [BL, C], F32)
    inp("ident", [P, P], BF)
    t_out = nc.dram_tensor("probs", [S, BL, C], F32, kind="ExternalOutput")
    with tile.TileContext(nc) as tc:
        with ExitStack() as ctx:
            _body(ctx, tc, t_in, t_out)
    nc.finalize()
    return nc


def _prep_shared(W_feat, W_hid, b_hid, w_score, W_ih, W_hh, b_ih, b_hh, W_gen, b_gen):
    bf = lambda x: np.ascontiguousarray(x, dtype=BF16)
    d = {}
    wf = W_feat.astype(np.float32)  # [H, I]
    d["wfeat"] = bf(wf.T.reshape(IQ, P, HQ, P))  # wfeat[iq][i][hq][h] = W_feat.T[i,h]
    # x.T rows: 0..511 ctx, 512..608 onehot, row 609 -> ones (bias row)
    wihT = np.zeros((KX * P, G4), np.float32)
    wihT[: I + C] = W_ih.astype(np.float32).T
    wihT[I + C] = b_ih.astype(np.float32) + b_hh.astype(np.float32)
    d["wih"] = bf(wihT.reshape(KX, P, G4))
    d["whh"] = bf(W_hh.astype(np.float32).T.reshape(HQ, P, G4))
    d["whid"] = bf(W_hid.astype(np.float32).T.reshape(HQ, P, H))
    d["wgen"] = bf(W_gen.astype(np.float32).T.reshape(HQ, P, C))
    wsc = np.zeros((HQ, P, 129), np.float32)
    wsc[:, :, 64] = w_score.astype(np.float32).reshape(HQ, P)
    d["wsc"] = bf(wsc)
    d["bhid"] = np.ascontiguousarray(b_hid.astype(np.float32).reshape(HQ, P, 1))
    d["bgen"] = np.ascontiguousarray(
        np.broadcast_to(b_gen.astype(np.float32), (BL, C)))
    d["ident"] = bf(np.eye(P))
    return d


def kernel(batch_H, text, W_feat, W_hid, b_hid, w_score, W_ih, W_hh, b_ih, b_hh,
           W_gen, b_gen, _trace=False):
    batch_H = np.asarray(batch_H, dtype=np.float32)
    text = np.asarray(text)
    shared = _prep_shared(W_feat, W_hid, b_hid, w_score, W_ih, W_hh, b_ih, b_hh,
                          W_gen, b_gen)
    oh = np.zeros((B, S, C), np.float32)
    oh[np.arange(B)[:, None], np.arange(S)[None, :], text.astype(np.int64)] = 1.0

    in_maps = []
    for c in range(NC):
        bh = batch_H[c * BL : (c + 1) * BL]          # [BL, T, I]
        m = dict(shared)
        m["bh_nat"] = np.ascontiguousarray(bh.transpose(1, 0, 2), dtype=BF16)
        m["bht"] = np.ascontiguousarray(
            bh.transpose(2, 0, 1).reshape(IQ, P, BL, T), dtype=BF16)
        ohc = np.zeros((S, P, BL), np.float32)
        ohc[:, :C, :] = oh[c * BL : (c + 1) * BL].transpose(1, 2, 0)
        ohc[:, 97, :] = 1.0      # ones row -> multiplies bias row of wih
        m["oht"] = np.ascontiguousarray(ohc, dtype=BF16)
        in_maps.append(m)

    runner, in_names, n_params = _get_runner()
    per_core = [[np.asarray(m[name]) for name in in_names[:n_params]] for m in in_maps]
    concat_in = [np.concatenate([per_core[c][i] for c in range(NC)], axis=0)
                 for i in range(n_params)]
    zeros = [np.zeros((NC * S, BL, C), np.float32)]
    _t0 = _time.perf_counter()
    out_arrs = runner(*concat_in, *zeros)
    probs_all = np.asarray(out_arrs[0]).reshape(NC, S, BL, C)
    kernel._last_exec_s = _time.perf_counter() - _t0
    out = np.empty((B, S, C), np.float32)
    for c in range(NC):
        out[c * BL : (c + 1) * BL] = probs_all[c].transpose(1, 0, 2)
    return out


@functools.cache
def _get_runner():
    import jax
    from jax.sharding import Mesh, PartitionSpec
    from jax.experimental.shard_map import shard_map
    from concourse import bass2jax
    from concourse.bass2jax import _bass_exec_p, install_neuronx_cc_hook, partition_id_tensor
    import concourse.mybir as mb

    install_neuronx_cc_hook()
    nc = build()
    partition_name = nc.partition_id_tensor.name if nc.partition_id_tensor else None
    in_names, out_names, out_avals, zero_shapes = [], [], [], []
    for alloc in nc.m.functions[0].allocations:
        if not isinstance(alloc, mb.MemoryLocationSet):
            continue
        name = alloc.memorylocations[0].name
        if alloc.kind == "ExternalInput":
            if name != partition_name:
                in_names.append(name)
        elif alloc.kind == "ExternalOutput":
            out_names.append(name)
            out_avals.append(jax.core.ShapedArray(tuple(alloc.tensor_shape),
                                                  mb.dt.np(alloc.dtype)))
    n_params = len(in_names)
    n_outs = len(out_avals)
    in_names = in_names + out_names
    if partition_name is not None:
        in_names.append(partition_name)
    donate = tuple(range(n_params, n_params + n_outs))

    def _body(*args):
        operands = list(args)
        if partition_name is not None:
            operands.append(partition_id_tensor())
        outs = _bass_exec_p.bind(
            *operands, out_avals=tuple(out_avals), in_names=tuple(in_names),
            out_names=tuple(out_names), lowering_input_output_aliases=(),
            sim_require_finite=True, sim_require_nnan=True, nc=nc)
        return tuple(outs)

    devices = jax.devices()[:NC]
    mesh = Mesh(np.asarray(devices), ("core",))
    in_specs = (PartitionSpec("core"),) * (n_params + n_outs)
    out_specs = (PartitionSpec("core"),) * n_outs
    runner = jax.jit(shard_map(_body, mesh=mesh, in_specs=in_specs,
                               out_specs=out_specs, check_rep=False),
                     donate_argnums=donate, keep_unused=True)
    return runner, in_names, n_params


# revision 14
# speedup vs baseline: 1.9801x; 1.0902x over previous
import sys, functools, time as _time
sys.path.insert(0, "/opt/trn_rl_repo")

import numpy as np
import ml_dtypes
from contextlib import ExitStack

import concourse.bass as bass
import concourse.tile as tile
from concourse import mybir, bacc
from concourse.bass_utils import run_bass_kernel_spmd

BF16 = ml_dtypes.bfloat16
F32 = mybir.dt.float32
BF = mybir.dt.bfloat16
AF = mybir.ActivationFunctionType

B, T, I, H, C, S = 512, 128, 512, 512, 97, 26
NC = 8
BL = B // NC  # 64 batch rows per core
P = 128
HQ = H // P  # 4
IQ = I // P  # 4
KX = 5       # x-contraction k-tiles: 4 (ctx) + 1 (onehot+bias)
G4 = 4 * H   # 2048


def _body(ctx, tc, t_in, t_out):
    nc = tc.nc
    sing = ctx.enter_context(tc.tile_pool(name="sing", bufs=1))
    work = ctx.enter_context(tc.tile_pool(name="work", bufs=2))
    pw = ctx.enter_context(tc.tile_pool(name="pw", bufs=1))
    psG = ctx.enter_context(tc.tile_pool(name="psG", bufs=1, space="PSUM"))
    psM = ctx.enter_context(tc.tile_pool(name="psM", bufs=3, space="PSUM"))

    # ---------- resident SBUF tensors ----------
    sb_bh = sing.tile([P, BL, I], BF)          # batch_H as [t, b, i]
    nc.sync.dma_start(sb_bh, t_in["bh_nat"][:])
    sb_wih = sing.tile([P, KX, G4], BF)
    for k in range(KX):
        nc.sync.dma_start(sb_wih[:, k, :], t_in["wih"][k])
    sb_whh = sing.tile([P, HQ, G4], BF)
    for k in range(HQ):
        nc.sync.dma_start(sb_whh[:, k, :], t_in["whh"][k])
    sb_whid = sing.tile([P, HQ, H], BF)
    for k in range(HQ):
        nc.sync.dma_start(sb_whid[:, k, :], t_in["whid"][k])
    sb_wgen = sing.tile([P, HQ, C], BF)
    for k in range(HQ):
        nc.sync.dma_start(sb_wgen[:, k, :], t_in["wgen"][k])
    sb_wsc = sing.tile([P, HQ, 129], BF)
    for k in range(HQ):
        nc.sync.dma_start(sb_wsc[:, k, :], t_in["wsc"][k])
    sb_bhid = sing.tile([P, HQ], F32)
    for k in range(HQ):
        nc.sync.dma_start(sb_bhid[:, k : k + 1], t_in["bhid"][k])
    sb_bgen = sing.tile([BL, C], F32)
    nc.sync.dma_start(sb_bgen, t_in["bgen"][:])
    sb_id = sing.tile([P, P], BF)
    nc.sync.dma_start(sb_id, t_in["ident"][:])

    sb_feat = sing.tile([P, BL, HQ, T], BF)    # feat.T: [h_in_q, (b, hq, t)]
    adiag = sing.tile([P, 65 * BL + 65], BF)   # alpha diag (t on partitions)
    nc.vector.memset(adiag, 0.0)
    hT = sing.tile([P, HQ, BL], BF)            # h.T state
    nc.vector.memset(hT, 0.0)
    c_sb = sing.tile([BL, H], F32)             # c state, natural
    nc.vector.memset(c_sb, 0.0)

    # ---------- feat precompute: feat.T[h,(b,t)] = W_feat @ batch_H.T ----------
    # bht dram: [iq, 128, BL, T]; chunks of 4 b's (512 cols)
    NB4 = BL // 4
    with tc.tile_pool(name="feed", bufs=1) as feed:
        sb_wfeat = pw.tile([P, IQ, HQ, P], BF, tag="pw_i")
        for k in range(IQ):
            nc.sync.dma_start(sb_wfeat[:, k, :, :], t_in["wfeat"][k])
        for cb in range(NB4):
            rhs = feed.tile([P, IQ, 4, T], BF, tag="bht")
            for k in range(IQ):
                nc.sync.dma_start(rhs[:, k, :, :], t_in["bht"][k, :, 4 * cb : 4 * cb + 4, :])
            for hq in range(HQ):
                ps = psM.tile([P, 4, T], F32, tag="sm")
                for k in range(IQ):
                    nc.tensor.matmul(ps, lhsT=sb_wfeat[:, k, hq, :], rhs=rhs[:, k, :, :],
                                     start=(k == 0), stop=(k == IQ - 1))
                nc.scalar.activation(sb_feat[:, 4 * cb : 4 * cb + 4, hq, :], ps,
                                     AF.Identity, bias=sb_bhid[:, hq : hq + 1], scale=1.0)

    # ---------- scan over S steps ----------
    for s in range(S):
        # hp.T[h,b] = W_hid @ h  (+ b_hid folded into feat)
        hp = work.tile([P, HQ, BL], F32, tag="hp")
        for hq in range(HQ):
            ps = psM.tile([P, BL], F32, tag="sm")
            for k in range(HQ):
                nc.tensor.matmul(ps, lhsT=sb_whid[:, k, hq * P : (hq + 1) * P],
                                 rhs=hT[:, k, :], start=(k == 0), stop=(k == HQ - 1))
            nc.vector.tensor_copy(hp[:, hq, :], ps)

        # tanh(feat + hp) and score e[b,t] accumulated via diagonal trick
        e_ps = psM.tile([BL, T], F32, tag="sm")
        for b in range(BL):
            th = work.tile([P, HQ, T], BF, tag="tanh")
            for hq in range(HQ):
                nc.scalar.activation(th[:, hq, :], sb_feat[:, b, hq, :], AF.Tanh,
                                     bias=hp[:, hq, b : b + 1], scale=1.0)
                nc.tensor.matmul(e_ps, lhsT=sb_wsc[:, hq, 64 - b : 128 - b],
                                 rhs=th[:, hq, :],
                                 start=(b == 0 and hq == 0),
                                 stop=(b == BL - 1 and hq == HQ - 1))

        # softmax over t (free dim)
        mx = work.tile([BL, 1], F32, tag="mx")
        nc.vector.tensor_reduce(mx, e_ps, axis=mybir.AxisListType.X, op=mybir.AluOpType.max)
        nmx = work.tile([BL, 1], F32, tag="nmx")
        nc.vector.tensor_scalar_mul(nmx, mx, -1.0)
        p_sb = work.tile([BL, T], F32, tag="p")
        ssum = work.tile([BL, 1], F32, tag="ssum")
        nc.scalar.activation(p_sb, e_ps, AF.Exp, bias=nmx, scale=1.0, accum_out=ssum)
        rs = work.tile([BL, 1], F32, tag="rs")
        nc.vector.reciprocal(rs, ssum)
        a_bf = work.tile([BL, T], BF, tag="abf")
        nc.vector.tensor_scalar_mul(a_bf, p_sb, rs)
        # alpha.T into diag tile (col 65*b holds alpha[:, b])
        tp = psM.tile([P, BL], BF, tag="sm")
        nc.tensor.transpose(tp, a_bf, sb_id[:BL, :BL])
        ad_v = adiag[:, 0 : 65 * BL].rearrange("p (k r) -> p k r", r=65)
        nc.vector.tensor_copy(ad_v[:, :, 0], tp)

        # ctx[b,i] accumulated via diagonal trick -> psum [BL, I]
        ctx_ps = psM.tile([BL, I], F32, tag="sm")
        for b in range(BL):
            nc.tensor.matmul(ctx_ps, lhsT=adiag[:, 64 * b : 64 * b + 64],
                             rhs=sb_bh[:, b, :], start=(b == 0), stop=(b == BL - 1))
        ctx_bf = work.tile([BL, I], BF, tag="ctxbf")
        nc.vector.tensor_copy(ctx_bf, ctx_ps)

        # x.T assembly: [ctx.T tiles; onehot+bias row tile]
        xT = work.tile([P, KX, BL], BF, tag="xT")
        for k in range(IQ):
            tp2 = psM.tile([P, BL], BF, tag="sm")
            nc.tensor.transpose(tp2, ctx_bf[:, k * P : (k + 1) * P], sb_id[:BL, :BL])
            nc.vector.tensor_copy(xT[:, k, :], tp2)
        oht_t = work.tile([P, BL], BF, tag="oht")
        nc.sync.dma_start(oht_t, t_in["oht"][s])
        nc.vector.tensor_copy(xT[:, 4, :], oht_t)

        # gates = x @ W_ih.T + h @ W_hh.T (+biases folded) -> psum [BL, 4H]
        g_ps = psG.tile([BL, G4], F32, tag="g")
        for k in range(KX):
            for nq in range(4):
                nc.tensor.matmul(g_ps[:, nq * H : (nq + 1) * H], lhsT=xT[:, k, :],
                                 rhs=sb_wih[:, k, nq * H : (nq + 1) * H],
                                 start=(k == 0), stop=False)
        for k in range(HQ):
            for nq in range(4):
                nc.tensor.matmul(g_ps[:, nq * H : (nq + 1) * H], lhsT=hT[:, k, :],
                                 rhs=sb_whh[:, k, nq * H : (nq + 1) * H],
                                 start=False, stop=(k == HQ - 1))

        # LSTM pointwise (gate order i,f,g,o)
        i_s = pw.tile([BL, H], F32, tag="pw_i")
        f_s = pw.tile([BL, H], F32, tag="pw_f")
        g_t = pw.tile([BL, H], F32, tag="pw_g")
        o_s = pw.tile([BL, H], F32, tag="pw_o")
        nc.scalar.activation(i_s, g_ps[:, 0:H], AF.Sigmoid)
        nc.scalar.activation(f_s, g_ps[:, H : 2 * H], AF.Sigmoid)
        nc.scalar.activation(g_t, g_ps[:, 2 * H : 3 * H], AF.Tanh)
        nc.scalar.activation(o_s, g_ps[:, 3 * H : 4 * H], AF.Sigmoid)
        nc.vector.tensor_mul(f_s, f_s, c_sb)
        nc.vector.tensor_mul(i_s, i_s, g_t)
        nc.vector.tensor_add(c_sb, f_s, i_s)
        nc.scalar.activation(g_t, c_sb, AF.Tanh)
        h_bf = work.tile([BL, H], BF, tag="hbf")
        nc.vector.tensor_mul(h_bf, o_s, g_t)
        # h.T for next step + generator
        for k in range(HQ):
            tp3 = psM.tile([P, BL], BF, tag="sm")
            nc.tensor.transpose(tp3, h_bf[:, k * P : (k + 1) * P], sb_id[:BL, :BL])
            nc.vector.tensor_copy(hT[:, k, :], tp3)

        # generator: probs_s = h @ W_gen.T + b_gen
        pr_ps = psM.tile([BL, C], F32, tag="sm")
        for k in range(HQ):
            nc.tensor.matmul(pr_ps, lhsT=hT[:, k, :], rhs=sb_wgen[:, k, :],
                             start=(k == 0), stop=(k == HQ - 1))
        pr_sb = work.tile([BL, C], F32, tag="prsb")
        nc.vector.tensor_add(pr_sb, pr_ps, sb_bgen)
        nc.sync.dma_start(t_out[s], pr_sb[:])


@functools.cache
def build():
    nc = bacc.Bacc("TRN2", target_bir_lowering=False)
    t_in = {}
    def inp(name, shape, dt):
        t_in[name] = nc.dram_tensor(name, shape, dt, kind="ExternalInput")
    inp("bh_nat", [P, BL, I], BF)
    inp("bht", [IQ, P, BL, T], BF)
    inp("wfeat", [IQ, P, HQ, P], BF)
    inp("wih", [KX, P, G4], BF)
    inp("whh", [HQ, P, G4], BF)
    inp("whid", [HQ, P, H], BF)
    inp("wgen", [HQ, P, C], BF)
    inp("wsc", [HQ, P, 129], BF)
    inp("oht", [S, P, BL], BF)
    inp("bhid", [HQ, P, 1], F32)
    inp("bgen", [BL, C], F32)
    inp("ident", [P, P], BF)
    t_out = nc.dram_tensor("probs", [S, BL, C], F32, kind="ExternalOutput")
    with tile.TileContext(nc) as tc:
        with ExitStack() as ctx:
            _body(ctx, tc, t_in, t_out)
    nc.finalize()
    return nc


def _prep_shared(W_feat, W_hid, b_hid, w_score, W_ih, W_hh, b_ih, b_hh, W_gen, b_gen):
    bf = lambda x: np.ascontiguousarray(x, dtype=BF16)
    d = {}
    wf = W_feat.astype(np.float32)  # [H, I]
    d["wfeat"] = bf(wf.T.reshape(IQ, P, HQ, P))  # wfeat[iq][i][hq][h] = W_feat.T[i,h]
    # x.T rows: 0..511 ctx, 512..608 onehot, row 609 -> ones (bias row)
    wihT = np.zeros((KX * P, G4), np.float32)
    wihT[: I + C] = W_ih.astype(np.float32).T
    wihT[I + C] = b_ih.astype(np.float32) + b_hh.astype(np.float32)
    d["wih"] = bf(wihT.reshape(KX, P, G4))
    d["whh"] = bf(W_hh.astype(np.float32).T.reshape(HQ, P, G4))
    d["whid"] = bf(W_hid.astype(np.float32).T.reshape(HQ, P, H))
    d["wgen"] = bf(W_gen.astype(np.float32).T.reshape(HQ, P, C))
    wsc = np.zeros((HQ, P, 129), np.float32)
    wsc[:, :, 64] = w_score.astype(np.float32).reshape(HQ, P)
    d["wsc"] = bf(wsc)
    d["bhid"] = np.ascontiguousarray(b_hid.astype(np.float32).reshape(HQ, P, 1))
    d["bgen"] = np.ascontiguousarray(
        np.broadcast_to(b_gen.astype(np.float32), (BL, C)))
    d["ident"] = bf(np.eye(P))
    return d


def kernel(batch_H, text, W_feat, W_hid, b_hid, w_score, W_ih, W_hh, b_ih, b_hh,
           W_gen, b_gen, _trace=False):
    batch_H = np.asarray(batch_H, dtype=np.float32)
    text = np.asarray(text)
    shared = _prep_shared(W_feat, W_hid, b_hid, w_score, W_ih, W_hh, b_ih, b_hh,
                          W_gen, b_gen)
    oh = np.zeros((B, S, C), np.float32)
    oh[np.arange(B)[:, None], np.arange(S)[None, :], text.astype(np.int64)] = 1.0

    in_maps = []
    for c in range(NC):
        bh = batch_H[c * BL : (c + 1) * BL]          # [BL, T, I]
        m = dict(shared)
        m["bh_nat"] = np.ascontiguousarray(bh.transpose(1, 0, 2), dtype=BF16)
        m["bht"] = np.ascontiguousarray(
            bh.transpose(2, 0, 1).reshape(IQ, P, BL, T), dtype=BF16)
        ohc = np.zeros((S, P, BL), np.float32)
        ohc[:, :C, :] = oh[c * BL : (c + 1) * BL].transpose(1, 2, 0)
        ohc[:, 97, :] = 1.0      # ones row -> multiplies bias row of wih
        m["oht"] = np.ascontiguousarray(ohc, dtype=BF16)
        in_maps.append(m)

    runner, in_names, n_params = _get_runner()
    per_core = [[np.asarray(m[name]) for name in in_names[:n_params]] for m in in_maps]
    concat_in = [np.concatenate([per_core[c][i] for c in range(NC)], axis=0)
                 for i in range(n_params)]
    zeros = [np.zeros((NC * S, BL, C), np.float32)]
    _t0 = _time.perf_counter()
    out_arrs = runner(*concat_in, *zeros)
    probs_all = np.asarray(out_arrs[0]).reshape(NC, S, BL, C)
    kernel._last_exec_s = _time.perf_counter() - _t0
    out = np.empty((B, S, C), np.float32)
    for c in range(NC):
        out[c * BL : (c + 1) * BL] = probs_all[c].transpose(1, 0, 2)
    return out


@functools.cache
def _get_runner():
    import jax
    from jax.sharding import Mesh, PartitionSpec
    from jax.experimental.shard_map import shard_map
    from concourse import bass2jax
    from concourse.bass2jax import _bass_exec_p, install_neuronx_cc_hook, partition_id_tensor
    import concourse.mybir as mb

    install_neuronx_cc_hook()
    nc = build()
    partition_name = nc.partition_id_tensor.name if nc.partition_id_tensor else None
    in_names, out_names, out_avals, zero_shapes = [], [], [], []
    for alloc in nc.m.functions[0].allocations:
        if not isinstance(alloc, mb.MemoryLocationSet):
            continue
        name = alloc.memorylocations[0].name
        if alloc.kind == "ExternalInput":
            if name != partition_name:
                in_names.append(name)
        elif alloc.kind == "ExternalOutput":
            out_names.append(name)
            out_avals.append(jax.core.ShapedArray(tuple(alloc.tensor_shape),
                                                  mb.dt.np(alloc.dtype)))
    n_params = len(in_names)
    n_outs = len(out_avals)
    in_names = in_names + out_names
    if partition_name is not None:
        in_names.append(partition_name)
    donate = tuple(range(n_params, n_params + n_outs))

    def _body(*args):
        operands = list(args)
        if partition_name is not None:
            operands.append(partition_id_tensor())
        outs = _bass_exec_p.bind(
            *operands, out_avals=tuple(out_avals), in_names=tuple(in_names),
            out_names=tuple(out_names), lowering_input_output_aliases=(),
            sim_require_finite=True, sim_require_nnan=True, nc=nc)
        return tuple(outs)

    devices = jax.devices()[:NC]
    mesh = Mesh(np.asarray(devices), ("core",))
    in_specs = (PartitionSpec("core"),) * (n_params + n_outs)
    out_specs = (PartitionSpec("core"),) * n_outs
    runner = jax.jit(shard_map(_body, mesh=mesh, in_specs=in_specs,
                               out_specs=out_specs, check_rep=False),
                     donate_argnums=donate, keep_unused=True)
    return runner, in_names, n_params
